# revision 1
# baseline (speedup 1.0000x reference)
"""Trainium2 Bass kernel for nn_AugmentedODE (B=64, N=P=512), 8-core data parallel.

Per batch the reference computes (7 matmuls of 512^3):
    Omega   = 0.5*(A - A^T)
    du      = u @ Omega + G - u @ (u^T G)
    S       = lam @ G^T
    dlam    = lam @ A + (S + S^T) @ u

Restructured to 5 fp32r matmuls + 3 PE transpose sets per batch:
    UTG = u^T G                      (native:   lhsT=u,    rhs=G)
    W   = 0.5*(A - A^T) - UTG        (DVE; A^T via fp32r PE transpose, from PSUM)
    du  = u @ W + G                  (lhsT=u^T, rhs=W; +G fused in PSUM->SBUF add)
    S   = lam @ G^T                  (lhsT=lam^T, rhs=G^T via fp32r PE transpose)
    C   = S + S^T                    (S^T via PE transpose accumulated into S's PSUM)
    dlam= lam @ A + C @ u            (8 matmuls accumulated into one PSUM group;
                                      C is symmetric so native layout works)

u^T / lam^T are pre-transposed on the host (pure data movement; lam natural is
never needed, so lam^T costs no extra DMA, and u^T trades 1MB of DMA for 16 PE
transposes).  Streaming more host-transposed copies (A^T, G^T) was measured
slower: the per-core HBM path sustains only ~260-280 GB/s in-kernel, so the
7MB/batch of this config is the sweet spot against ~182us of PE work.
"""
import numpy as np

import concourse.bass as bass
import concourse.mybir as mybir
import concourse.tile as tile
from concourse import bacc
from concourse.bass_utils import run_bass_kernel_spmd
from concourse.masks import make_identity

F32 = mybir.dt.float32
F32R = mybir.dt.float32r
AOP = mybir.AluOpType

B, N, P = 64, 512, 512
NCORES = 8
BLOC = B // NCORES          # batches per core
KB = 4                      # 512 = 4 k-blocks of 128
CH = 4                      # 4 output chunks of 128 rows


def _build_nc():
    nc = bacc.Bacc("TRN2", target_bir_lowering=False, debug=False,
                   num_devices=NCORES)

    d_u = nc.declare_dram_parameter("u", [BLOC, N, P], F32R, isOutput=False)
    d_ut = nc.declare_dram_parameter("ut", [BLOC, P, N], F32R, isOutput=False)
    d_g = nc.declare_dram_parameter("g", [BLOC, N, P], F32R, isOutput=False)
    d_a = nc.declare_dram_parameter("a", [BLOC, P, P], F32R, isOutput=False)
    d_lamt = nc.declare_dram_parameter("lamt", [BLOC, P, N], F32R, isOutput=False)
    d_du = nc.declare_dram_parameter("du", [BLOC, N, P], F32, isOutput=True)
    d_dlam = nc.declare_dram_parameter("dlam", [BLOC, N, P], F32, isOutput=True)

    with tile.TileContext(nc) as tc:
        with (
            tc.tile_pool(name="const", bufs=1) as constp,
            tc.tile_pool(name="ins", bufs=2) as insp,
            tc.tile_pool(name="mid", bufs=1) as midp,
            tc.tile_pool(name="outs", bufs=2) as outsp,
            tc.tile_pool(name="psum", bufs=8, space="PSUM") as psum,
        ):
            ident = constp.tile([128, 128], F32)
            make_identity(nc, ident[:])
            identr = constp.tile([128, 128], F32R)
            nc.vector.tensor_copy(identr[:], ident[:])

            # HAM warm-up: ~5us of dummy matmuls during the head DMA wait so
            # the first real batch runs at 2.4GHz instead of the cold 1.2GHz
            warm_ps = psum.tile([128, 512], F32, tag="ps")
            wsrc = constp.tile([128, 512], F32R)
            nc.gpsimd.memset(wsrc[:].bitcast(F32), 0.0)
            for i in range(12):
                nc.tensor.matmul(warm_ps[:], identr[:], wsrc[:],
                                 start=True, stop=True)

            for b in range(BLOC):
                u_sb = insp.tile([128, KB, P], F32R, tag="u")
                ut_sb = insp.tile([128, KB, N], F32R, tag="ut")
                g_sb = insp.tile([128, KB, P], F32R, tag="g")
                a_sb = insp.tile([128, KB, P], F32R, tag="a")
                lamt_sb = insp.tile([128, KB, N], F32R, tag="lamt")
                # issue order ~ consumption order (g/a feed the PE transposes first)
                if b == 0:
                    g_r = d_g[b].rearrange("(k p) c -> p k c", p=128)
                    a_r = d_a[b].rearrange("(k p) c -> p k c", p=128)
                    nc.sync.dma_start(g_sb[:, 0:2], g_r[:, 0:2])
                    nc.scalar.dma_start(g_sb[:, 2:4], g_r[:, 2:4])
                    nc.sync.dma_start(a_sb[:, 0:2], a_r[:, 0:2])
                    nc.scalar.dma_start(a_sb[:, 2:4], a_r[:, 2:4])
                else:
                    nc.sync.dma_start(g_sb[:], d_g[b].rearrange("(k p) c -> p k c", p=128))
                    nc.sync.dma_start(a_sb[:], d_a[b].rearrange("(k p) c -> p k c", p=128))
                nc.sync.dma_start(u_sb[:], d_u[b].rearrange("(k p) c -> p k c", p=128))
                nc.sync.dma_start(lamt_sb[:], d_lamt[b].rearrange("(k p) c -> p k c", p=128))
                nc.sync.dma_start(ut_sb[:], d_ut[b].rearrange("(k p) c -> p k c", p=128))

                # ---- Gt via PE transpose: Gt[r][p, 128c:] = G[c-block, 128r:].T ----
                # fp32r transpose mode: 1.5 cycles/row vs 2.0 for fp32
                gt_sb = midp.tile([128, KB, N], F32R, tag="gt", bufs=2)
                for r in range(CH):
                    ps = psum.tile([128, P], F32R, tag="ps")
                    for c in range(KB):
                        nc.tensor.transpose(
                            ps[:, c * 128:(c + 1) * 128],
                            g_sb[:, c, r * 128:(r + 1) * 128],
                            identr[:],
                        )
                    nc.scalar.copy(gt_sb[:, r, :], ps[:])

                # ---- At via PE transpose (stays in PSUM, consumed by DVE) ----
                at_ps = []
                for r in range(CH):
                    ps = psum.tile([128, P], F32R, tag="ps")
                    for c in range(KB):
                        nc.tensor.transpose(
                            ps[:, c * 128:(c + 1) * 128],
                            a_sb[:, c, r * 128:(r + 1) * 128],
                            identr[:],
                        )
                    at_ps.append(ps)

                # ---- M1: UTG = u^T G ; W = 0.5*(A - At) - UTG (DVE) ----
                w1_sb = midp.tile([128, KB, P], F32, tag="w1")
                w_sb = midp.tile([128, KB, P], F32R, tag="w", bufs=2)
                for r in range(CH):
                    utg = psum.tile([128, P], F32, tag="ps")
                    for k in range(KB):
                        nc.tensor.matmul(utg[:], u_sb[:, k, r * 128:(r + 1) * 128],
                                         g_sb[:, k, :], start=(k == 0), stop=(k == KB - 1))
                    nc.vector.tensor_tensor(w1_sb[:, r, :], a_sb[:, r, :].bitcast(F32),
                                            at_ps[r][:].bitcast(F32), AOP.subtract)
                    nc.vector.scalar_tensor_tensor(w_sb[:, r, :], w1_sb[:, r, :], 0.5,
                                                   utg[:], AOP.mult, AOP.subtract)

                # ---- M5: S = lam @ G^T (group left open for S^T accumulation) ----
                s_ps = []
                s_sb = midp.tile([128, KB, N], F32R, tag="s")
                for r in range(CH):
                    ps = psum.tile([128, N], F32, tag="ps")
                    for k in range(KB):
                        nc.tensor.matmul(ps[:], lamt_sb[:, k, r * 128:(r + 1) * 128],
                                         gt_sb[:, k, :], start=(k == 0), stop=False)
                    nc.scalar.copy(s_sb[:, r, :], ps[:])
                    s_ps.append(ps)

                # ---- M23: du = u @ W + G ----
                du_sb = outsp.tile([128, KB, P], F32, tag="du")
                for r in range(CH):
                    ps = psum.tile([128, P], F32, tag="ps")
                    for k in range(KB):
                        nc.tensor.matmul(ps[:], ut_sb[:, k, r * 128:(r + 1) * 128],
                                         w_sb[:, k, :], start=(k == 0), stop=(k == KB - 1))
                    nc.vector.tensor_tensor(du_sb[:, r, :], ps[:],
                                            g_sb[:, r, :].bitcast(F32), AOP.add)
                nc.sync.dma_start(d_du[b].rearrange("(k p) c -> p k c", p=128), du_sb[:])

                # ---- S^T accumulated into S's PSUM -> C = S + S^T ----
                coup_sb = midp.tile([128, KB, N], F32R, tag="coup")
                for r in range(CH):
                    for c in range(KB):
                        nc.tensor.matmul(
                            s_ps[r][:, c * 128:(c + 1) * 128].bitcast(F32R),
                            s_sb[:, c, r * 128:(r + 1) * 128],
                            identr[:],
                            is_transpose=True,
                            start=False, stop=(c == KB - 1),
                        )
                for r in range(CH):
                    nc.vector.tensor_copy(coup_sb[:, r, :], s_ps[r][:])

                # ---- M4+M7: dlam = lam @ A + C @ u ----
                # all M4 groups first: their 16 matmuls hide the DVE coupling
                # copies that M7 needs
                dlam_sb = outsp.tile([128, KB, P], F32, tag="dlam")
                dlam_ps = []
                for r in range(CH):
                    ps = psum.tile([128, P], F32, tag="ps")
                    for k in range(KB):
                        nc.tensor.matmul(ps[:], lamt_sb[:, k, r * 128:(r + 1) * 128],
                                         a_sb[:, k, :], start=(k == 0), stop=False)
                    dlam_ps.append(ps)
                for r in range(CH):
                    ps = dlam_ps[r]
                    for k in range(KB):
                        nc.tensor.matmul(ps[:], coup_sb[:, k, r * 128:(r + 1) * 128],
                                         u_sb[:, k, :], start=False, stop=(k == KB - 1))
                    if b == BLOC - 1:
                        # tail: alternate copy engines and output rings so the
                        # last batch's copies and DMAs drain in parallel
                        if r % 2 == 0:
                            nc.vector.tensor_copy(dlam_sb[:, r, :], ps[:])
                            nc.sync.dma_start(
                                d_dlam[b].rearrange("(k p) c -> p k c", p=128)[:, r],
                                dlam_sb[:, r, :])
                        else:
                            nc.scalar.copy(dlam_sb[:, r, :], ps[:])
                            nc.scalar.dma_start(
                                d_dlam[b].rearrange("(k p) c -> p k c", p=128)[:, r],
                                dlam_sb[:, r, :])
                    else:
                        nc.scalar.copy(dlam_sb[:, r, :], ps[:])
                if b < BLOC - 1:
                    nc.scalar.dma_start(d_dlam[b].rearrange("(k p) c -> p k c", p=128),
                                        dlam_sb[:])

    nc.compile()
    return nc


_NC = None


def _make_in_maps(u, lam, A, G):
    u = np.ascontiguousarray(u, dtype=np.float32)
    lam = np.ascontiguousarray(lam, dtype=np.float32)
    A = np.ascontiguousarray(A, dtype=np.float32)
    G = np.ascontiguousarray(G, dtype=np.float32)
    ut = np.ascontiguousarray(np.swapaxes(u, 1, 2))
    lamt = np.ascontiguousarray(np.swapaxes(lam, 1, 2))

    in_maps = []
    for c in range(NCORES):
        sl = slice(c * BLOC, (c + 1) * BLOC)
        in_maps.append({
            "u": u[sl], "ut": ut[sl], "g": G[sl], "a": A[sl], "lamt": lamt[sl],
        })
    return in_maps


def kernel(u, lam, A, G, t=None, **_ignored):
    global _NC
    if _NC is None:
        _NC = _build_nc()
    nc = _NC

    in_maps = _make_in_maps(u, lam, A, G)
    res = run_bass_kernel_spmd(nc, in_maps, list(range(NCORES)))
    du = np.concatenate([res.results[c]["du"] for c in range(NCORES)], axis=0)
    dlam = np.concatenate([res.results[c]["dlam"] for c in range(NCORES)], axis=0)
    return du, dlam



# revision 2
# speedup vs baseline: 1.2494x; 1.2494x over previous
"""Trainium2 Bass kernel for nn_AugmentedODE (B=64, N=P=512), 8-core data parallel.

Per batch the reference computes (7 matmuls of 512^3):
    Omega   = 0.5*(A - A^T)
    du      = u @ Omega + G - u @ (u^T G)
    S       = lam @ G^T
    dlam    = lam @ A + (S + S^T) @ u

Restructured to 5 matmuls + 1 PE transpose set per batch:
    UTG = u^T G                      (bf16:  lhsT=u,    rhs=G)
    W   = 0.5*(A - A^T) - UTG        (DVE; A - A^T precomputed host-side, fp8)
    du  = u @ W + G                  (bf16:  lhsT=u^T,  rhs=W; +G fused in DVE)
    S   = lam @ G^T                  (bf16:  lhsT=lam^T, rhs=G^T)
    C   = S + S^T                    (bf16 PE transpose + DVE add)
    dlam= lam @ A + C @ u            (lam@A in fp8 DoubleRow at 2x rate; C@u bf16;
                                      both accumulated into one PSUM group)

Rel-err budget is 2e-2 (Frobenius); measured ~3.3e-3 for this mix.  The four
magnitude-dominant matmuls (UTG, u@W, S, C@u) stay bf16; only lam@A (~3% of
|dlam|) and the skew term (~5% of |W|) are fp8.

All operands are pre-packed on the host into the exact SBUF layout
([128 partitions, kblock, 512] with k-blocks contiguous per partition) and
concatenated into three blobs per batch, so every DMA line is 4-12KB
contiguous on both sides.  Outputs are written bf16 and upcast on the host.
"""
import numpy as np
import ml_dtypes

import concourse.bass as bass
import concourse.mybir as mybir
import concourse.tile as tile
from concourse import bacc
from concourse.bass_utils import run_bass_kernel_spmd
from concourse.masks import make_identity

F32 = mybir.dt.float32
F32R = mybir.dt.float32r
BF16 = mybir.dt.bfloat16
F8 = mybir.dt.float8e4
AOP = mybir.AluOpType
DR = mybir.MatmulPerfMode.DoubleRow

NP_BF16 = ml_dtypes.bfloat16
NP_F8 = ml_dtypes.float8_e4m3

B, N, P = 64, 512, 512
NCORES = 8
BLOC = B // NCORES          # batches per core
KB = 4                      # 512 = 4 k-blocks of 128
CH = 4                      # 4 output chunks of 128 rows

M4_DOUBLE_ROW = True        # lam@A as fp8 DoubleRow (2x PE rate)


def _build_nc():
    nc = bacc.Bacc("TRN2", target_bir_lowering=False, debug=False,
                   num_devices=NCORES)

    # in1: u(0:4) | g(4:8);  in2: ut(0:4) | gt(4:8) | lamt(8:12)
    # in8: amat=A-A^T(0:4) | a(4:8), fp8e4
    d_in1 = nc.declare_dram_parameter("in1", [BLOC, 128, 2 * KB, P], BF16,
                                      isOutput=False)
    d_in2 = nc.declare_dram_parameter("in2", [BLOC, 128, 3 * KB, P], BF16,
                                      isOutput=False)
    d_in8 = nc.declare_dram_parameter("in8", [BLOC, 128, 2 * KB, P], F8,
                                      isOutput=False)
    d_du = nc.declare_dram_parameter("du", [BLOC, 128, KB, P], BF16,
                                     isOutput=True)
    d_dlam = nc.declare_dram_parameter("dlam", [BLOC, 128, KB, P], BF16,
                                       isOutput=True)

    with tile.TileContext(nc) as tc:
        with (
            tc.tile_pool(name="const", bufs=1) as constp,
            tc.tile_pool(name="ins", bufs=2) as insp,
            tc.tile_pool(name="mid", bufs=2) as midp,
            tc.tile_pool(name="outs", bufs=2) as outsp,
            tc.tile_pool(name="psum", bufs=6, space="PSUM") as psum,
        ):
            ident = constp.tile([128, 128], F32)
            make_identity(nc, ident[:])
            identb = constp.tile([128, 128], BF16)
            nc.vector.tensor_copy(identb[:], ident[:])

            # HAM warm-up: dummy matmuls during the head DMA wait so the
            # first real batch runs at full clock instead of the cold p-state
            warm_ps = psum.tile([128, 512], F32, tag="ps")
            wsrc = constp.tile([128, 512], BF16)
            nc.gpsimd.memset(wsrc[:].bitcast(F32), 0.0)
            for i in range(12):
                nc.tensor.matmul(warm_ps[:], identb[:], wsrc[:],
                                 start=True, stop=True)

            for b in range(BLOC):
                in1 = insp.tile([128, 2 * KB, P], BF16, tag="in1")
                in2 = insp.tile([128, 3 * KB, P], BF16, tag="in2")
                in8 = insp.tile([128, 2 * KB, P], F8, tag="in8")
                nc.sync.dma_start(in1[:], d_in1[b])
                nc.scalar.dma_start(in8[:], d_in8[b])
                nc.gpsimd.dma_start(in2[:], d_in2[b])

                Un = lambda k: in1[:, k]        # u natural, n-block k
                Gn = lambda k: in1[:, KB + k]   # g natural
                UT = lambda k: in2[:, k]        # u^T, q-block k
                GT = lambda k: in2[:, KB + k]   # g^T
                LT = lambda k: in2[:, 2 * KB + k]  # lam^T
                AM8 = lambda k: in8[:, k]       # A - A^T, q-block k
                A8 = lambda k: in8[:, KB + k]   # A natural, fp8

                # ---- M1: UTG = u^T G ; W = 0.5*amat - UTG (DVE) ----
                w_sb = midp.tile([128, KB, P], BF16, tag="w")
                for r in range(CH):
                    utg = psum.tile([128, P], F32, tag="ps")
                    for k in range(KB):
                        nc.tensor.matmul(utg[:], Un(k)[:, r * 128:(r + 1) * 128],
                                         Gn(k)[:], start=(k == 0), stop=(k == KB - 1))
                    nc.vector.scalar_tensor_tensor(w_sb[:, r], AM8(r)[:], 0.5,
                                                   utg[:], AOP.mult, AOP.subtract)

                # ---- M5: S = lam @ G^T ----
                s_sb = midp.tile([128, KB, N], BF16, tag="s")
                for r in range(CH):
                    ps = psum.tile([128, N], F32, tag="ps")
                    for k in range(KB):
                        nc.tensor.matmul(ps[:], LT(k)[:, r * 128:(r + 1) * 128],
                                         GT(k)[:], start=(k == 0), stop=(k == KB - 1))
                    nc.scalar.copy(s_sb[:, r], ps[:])

                # lam^T cast to fp8 for the DoubleRow lam@A
                if M4_DOUBLE_ROW:
                    lamt8 = midp.tile([128, KB, N], F8, tag="l8")
                    for r in range(CH):
                        if r % 2 == 0:
                            nc.vector.tensor_copy(lamt8[:, r], LT(r)[:])
                        else:
                            nc.scalar.copy(lamt8[:, r], LT(r)[:])

                # ---- M23: du = u @ W + G ----
                du_sb = outsp.tile([128, KB, P], BF16, tag="du")
                for r in range(CH):
                    ps = psum.tile([128, P], F32, tag="ps")
                    for k in range(KB):
                        nc.tensor.matmul(ps[:], UT(k)[:, r * 128:(r + 1) * 128],
                                         w_sb[:, k], start=(k == 0), stop=(k == KB - 1))
                    nc.vector.tensor_tensor(du_sb[:, r], ps[:], Gn(r)[:], AOP.add)
                nc.sync.dma_start(d_du[b], du_sb[:])

                # ---- C = S + S^T (bf16 PE transpose into PSUM + DVE add) ----
                coup_sb = midp.tile([128, KB, N], BF16, tag="coup")
                for r in range(CH):
                    tps = psum.tile([128, N], BF16, tag="tps", bufs=2)
                    for c in range(KB):
                        nc.tensor.transpose(tps[:, c * 128:(c + 1) * 128],
                                            s_sb[:, c, r * 128:(r + 1) * 128],
                                            identb[:])
                    nc.vector.tensor_tensor(coup_sb[:, r], tps[:], s_sb[:, r],
                                            AOP.add)

                # ---- M4+M7: dlam = lam @ A + C @ u ----
                dlam_sb = outsp.tile([128, KB, P], BF16, tag="dlam")
                for r in range(CH):
                    ps = psum.tile([128, P], F32, tag="ps")
                    if M4_DOUBLE_ROW:
                        for j in range(2):
                            nc.tensor.matmul(
                                ps[:],
                                lamt8[:, 2 * j:2 * j + 2, r * 128:(r + 1) * 128],
                                in8[:, KB + 2 * j:KB + 2 * j + 2],
                                perf_mode=DR, start=(j == 0), stop=False,
                                skip_group_check=True)
                    else:
                        for k in range(KB):
                            nc.tensor.matmul(ps[:], LT(k)[:, r * 128:(r + 1) * 128],
                                             A8(k)[:], start=(k == 0), stop=False)
                    for k in range(KB):
                        nc.tensor.matmul(ps[:], coup_sb[:, k, r * 128:(r + 1) * 128],
                                         Un(k)[:], start=False, stop=(k == KB - 1),
                                         skip_group_check=True)
                    if b == BLOC - 1:
                        # tail: alternate copy engines and DMA queues so the
                        # last batch's copies and stores drain in parallel
                        if r % 2 == 0:
                            nc.vector.tensor_copy(dlam_sb[:, r], ps[:])
                            nc.sync.dma_start(d_dlam[b][:, r], dlam_sb[:, r])
                        else:
                            nc.scalar.copy(dlam_sb[:, r], ps[:])
                            nc.scalar.dma_start(d_dlam[b][:, r], dlam_sb[:, r])
                    else:
                        nc.scalar.copy(dlam_sb[:, r], ps[:])
                if b < BLOC - 1:
                    nc.scalar.dma_start(d_dlam[b], dlam_sb[:])

    nc.compile()
    return nc


_NC = None


def _pack(x, dt):
    """[BLOC,512,512] -> [BLOC,128,4,512] in SBUF layout (partition-major)."""
    return np.ascontiguousarray(
        x.reshape(BLOC, KB, 128, P).transpose(0, 2, 1, 3).astype(dt))


def _unpack(y):
    """[BLOC,128,4,512] bf16 -> [BLOC,512,512] fp32."""
    return y.transpose(0, 2, 1, 3).reshape(BLOC, N, P).astype(np.float32)


def _make_in_maps(u, lam, A, G):
    u = np.asarray(u, dtype=np.float32)
    lam = np.asarray(lam, dtype=np.float32)
    A = np.asarray(A, dtype=np.float32)
    G = np.asarray(G, dtype=np.float32)

    in_maps = []
    for c in range(NCORES):
        sl = slice(c * BLOC, (c + 1) * BLOC)
        uc, lamc, Ac, Gc = u[sl], lam[sl], A[sl], G[sl]
        At = np.swapaxes(Ac, 1, 2)
        in1 = np.concatenate([_pack(uc, NP_BF16), _pack(Gc, NP_BF16)], axis=2)
        in2 = np.concatenate([_pack(np.swapaxes(uc, 1, 2), NP_BF16),
                              _pack(np.swapaxes(Gc, 1, 2), NP_BF16),
                              _pack(np.swapaxes(lamc, 1, 2), NP_BF16)], axis=2)
        in8 = np.concatenate([_pack(Ac - At, NP_F8), _pack(Ac, NP_F8)], axis=2)
        in_maps.append({"in1": in1, "in2": in2, "in8": in8})
    return in_maps


def kernel(u, lam, A, G, t=None, **_ignored):
    global _NC
    if _NC is None:
        _NC = _build_nc()
    nc = _NC

    in_maps = _make_in_maps(u, lam, A, G)
    res = run_bass_kernel_spmd(nc, in_maps, list(range(NCORES)))
    du = np.concatenate([_unpack(res.results[c]["du"]) for c in range(NCORES)],
                        axis=0)
    dlam = np.concatenate([_unpack(res.results[c]["dlam"])
                           for c in range(NCORES)], axis=0)
    return du, dlam
